# revision 6
# baseline (speedup 1.0000x reference)
"""Multi-head attention on 8 Trainium2 NeuronCores.

Problem: x[2, 2048, 1024] -> qkv proj (w_qkv [1024, 3072], 16 heads x 64) ->
softmax attention -> out proj (w_out [1024, 1024] + b_out).

Sharding: core c in 0..7 handles batch b = c // 4 and heads 4*(c%4) .. 4*(c%4)+3.
Each core computes a partial output projection over its 4 heads' slice and the
four cores of each batch group AllReduce(add) the partials (bias/4 folded in on
every core). Host gathers core 0 (batch 0) and core 4 (batch 1).

Per-core dataflow (all matmuls fp32r: ~1e-4 rel err, bf16-rate on the PE):
  qkT [512, 2048] = wqk.T @ xT        (d-on-partitions layout for Q and K)
  v   [2048, 260] = xT.T @ wv         (natural layout, +ones column per head)
  S_T [j, i] = kT.T-slices @ qT       (scores transposed, K=64, head pairs
                                       share the PE via base-partition 0/64)
  attn_T = exp(S_T * 0.125)           (ScalarE, scale folded into activation)
  O_T[h] [65, i] = v_aug.T @ attn_T   (row 64 = softmax denominator, free)
  o[h] = O_T[0:64] * (1/denom broadcast)  (PE K=1 broadcast + DVE multiply)
  y += o[h].T @ wout[h] (+ bias/4)    (K=64 per head, accumulated in PSUM)
  AllReduce(y) over the 4-core batch group, chunked to overlap.
"""

import numpy as np

N = 2048          # sequence length per batch
D = 1024          # model dim
DH = 64           # head dim
HPC = 4           # heads per core
NCORES = 8
SCALE = DH ** -0.5

_cached = {}


def _build_nc():
    from contextlib import ExitStack

    import concourse.bacc as bacc
    import concourse.mybir as mybir
    from concourse import tile

    f32 = mybir.dt.float32
    f32r = mybir.dt.float32r

    nc = bacc.Bacc(num_devices=NCORES)

    xT = nc.declare_dram_parameter("xT", [D, N], f32r, isOutput=False)
    wqk = nc.declare_dram_parameter("wqk", [D, 2 * HPC * DH], f32r, isOutput=False)
    wv = nc.declare_dram_parameter("wv", [D, HPC * DH], f32r, isOutput=False)
    wout = nc.declare_dram_parameter("wout", [HPC * DH, D], f32r, isOutput=False)
    bias = nc.declare_dram_parameter("bias", [1, D], f32r, isOutput=False)
    ones1 = nc.declare_dram_parameter("ones1", [1, 128], f32r, isOutput=False)
    quart = nc.declare_dram_parameter("quart", [1, 128], f32r, isOutput=False)
    y_out = nc.declare_dram_parameter("y", [N, D], f32, isOutput=True)

    KB = D // 128           # 8 contraction blocks for the projections
    JB = N // 128           # 16 key blocks
    ICH = N // 512          # 4 query chunks of 512
    GRP = 2                 # j-blocks per exp group (PSUM: [128, 1024] tiles)
    VW = DH + 1             # v columns per head incl. ones column

    with tile.TileContext(nc) as tc:
        ctx = ExitStack()
        with ctx:
            sb = ctx.enter_context(tc.tile_pool(name="sb", bufs=1))
            ps_mm = ctx.enter_context(tc.tile_pool(name="ps_mm", bufs=2, space="PSUM"))
            ps_s = ctx.enter_context(tc.tile_pool(name="ps_s", bufs=2, space="PSUM"))
            ps_o = ctx.enter_context(tc.tile_pool(name="ps_o", bufs=2, space="PSUM"))
            dram = ctx.enter_context(tc.tile_pool(name="dram", bufs=1, space="DRAM"))

            # persistent SBUF residents (~105 KB/partition)
            qk_sb = sb.tile([128, 4, N], f32r, tag="qk")
            v_sb = sb.tile([128, JB, HPC * VW], f32r, tag="v")
            o_sb = sb.tile([64, HPC, N], f32r, tag="o")
            wo_sb = sb.tile([64, HPC, D], f32r, tag="wo")
            bias_bc = sb.tile([128, D], f32, tag="bias_bc")
            bias_sb = sb.tile([1, D], f32r, tag="bias")
            ones_sb = sb.tile([1, 128], f32r, tag="ones1")
            quart_sb = sb.tile([1, 128], f32r, tag="quart")

            nc.sync.dma_start(out=bias_sb[:], in_=bias[:, :])
            nc.sync.dma_start(out=ones_sb[:], in_=ones1[:, :])
            nc.sync.dma_start(out=quart_sb[:], in_=quart[:, :])
            for h in range(HPC):
                nc.sync.dma_start(out=wo_sb[:, h, :], in_=wout[h * DH:(h + 1) * DH, :])

            # bias/4 broadcast to 128 partitions (PE K=1 matmul)
            for ch in range(D // 512):
                bps = ps_mm.tile([128, 512], f32, tag="mm", name=f"bps{ch}")
                nc.tensor.matmul(bps[:], quart_sb[:], bias_sb[:, ch * 512:(ch + 1) * 512],
                                 start=True, stop=True)
                nc.vector.tensor_copy(bias_bc[:, ch * 512:(ch + 1) * 512], bps[:])

            # ---- stage 1: projections (xT + weights live only here) ----
            with ExitStack() as s1:
                sb_x = s1.enter_context(tc.tile_pool(name="sb_x", bufs=1))
                xT_sb = sb_x.tile([128, KB, N], f32r, tag="xT")
                for kb in range(KB):
                    nc.sync.dma_start(out=xT_sb[:, kb, :], in_=xT[kb * 128:(kb + 1) * 128, :])
                wqk_sb = sb_x.tile([128, KB, 2 * HPC * DH], f32r, tag="wqk")
                nc.sync.dma_start(out=wqk_sb[:], in_=wqk[:, :].rearrange("(kb p) m -> p kb m", p=128))
                wv_sb = sb_x.tile([128, KB, HPC * DH], f32r, tag="wv")
                nc.sync.dma_start(out=wv_sb[:], in_=wv[:, :].rearrange("(kb p) m -> p kb m", p=128))

                # qkT: mb 0 = q heads 01 | mb 1 = q heads 23 | mb 2 = k heads 01 | mb 3 = k heads 23
                for mb in [0, 2, 1, 3]:
                    for ich in range(ICH):
                        mps = ps_mm.tile([128, 512], f32, tag="mm", name=f"mps{mb}_{ich}")
                        for kb in range(KB):
                            nc.tensor.matmul(
                                mps[:],
                                wqk_sb[:, kb, mb * 128:(mb + 1) * 128],
                                xT_sb[:, kb, ich * 512:(ich + 1) * 512],
                                start=(kb == 0), stop=(kb == KB - 1))
                        nc.vector.tensor_copy(qk_sb[:, mb, ich * 512:(ich + 1) * 512], mps[:])

                # v natural; ones columns = whatever the evictions don't overwrite
                nc.vector.memset(v_sb[:].bitcast(f32), 1.0)
                for jb in range(JB):
                    vps = ps_mm.tile([128, 256], f32, tag="mm", name=f"vps{jb}")
                    for kb in range(KB):
                        nc.tensor.matmul(
                            vps[:],
                            xT_sb[:, kb, jb * 128:(jb + 1) * 128],
                            wv_sb[:, kb, :],
                            start=(kb == 0), stop=(kb == KB - 1))
                    nc.vector.tensor_copy(
                        v_sb[:, jb, :].rearrange("p (h c) -> p h c", c=VW)[:, :, 0:DH],
                        vps[:].rearrange("p (h c) -> p h c", c=DH))

            # ---- attention (head pairs share the PE via base-partition 0/64) ----
            sb_attn = ctx.enter_context(tc.tile_pool(name="sb_attn", bufs=3))
            sb_work = ctx.enter_context(tc.tile_pool(name="sb_work", bufs=2))
            otmp_pool = ctx.enter_context(tc.tile_pool(name="otmp", bufs=2))
            rbc_pool = ctx.enter_context(tc.tile_pool(name="rbc", bufs=2))

            def attention_pair(pair):
                otmps = {}
                for s in range(2):
                    h = pair * 2 + s
                    otmps[h] = otmp_pool.tile([65, N], f32, tag="otmp", name=f"otmp{h}")
                for ich in range(ICH):
                    isl = slice(ich * 512, (ich + 1) * 512)
                    ops = {}
                    for s in range(2):
                        h = pair * 2 + s
                        ops[h] = ps_o.tile([65, 512], f32, tag="o", name=f"ops{h}_{ich}")
                    for g in range(JB // GRP):
                        for s in range(2):
                            h = pair * 2 + s
                            psl = slice(s * 64, s * 64 + 64)
                            st = ps_s.tile([128, GRP * 512], f32, tag="s", name=f"st{h}_{ich}_{g}")
                            for u in range(GRP):
                                jb = g * GRP + u
                                nc.tensor.matmul(
                                    st[:, u * 512:(u + 1) * 512],
                                    qk_sb[psl, 2 + pair, jb * 128:(jb + 1) * 128],
                                    qk_sb[psl, pair, isl],
                                    start=True, stop=True)
                            at = sb_attn.tile([128, GRP * 512], f32r, tag="attn", name=f"at{h}_{ich}_{g}")
                            nc.scalar.activation(at[:], st[:],
                                                 mybir.ActivationFunctionType.Exp,
                                                 scale=float(SCALE))
                            for u in range(GRP):
                                jb = g * GRP + u
                                nc.tensor.matmul(
                                    ops[h][:],
                                    v_sb[:, jb, h * VW:(h + 1) * VW],
                                    at[:, u * 512:(u + 1) * 512],
                                    start=(g == 0 and u == 0),
                                    stop=(g == JB // GRP - 1 and u == GRP - 1))
                    for s in range(2):
                        h = pair * 2 + s
                        nc.vector.tensor_copy(otmps[h][:, isl], ops[h][:])
                # normalize: reciprocal of denominator row, PE broadcast, DVE multiply
                for s in range(2):
                    h = pair * 2 + s
                    rden = sb_work.tile([1, N], f32r, tag="rden", name=f"rden{h}")
                    with nc.allow_low_precision(reason="f32r rounding of softmax denom recip"):
                        nc.vector.reciprocal(rden[:], otmps[h][64:65, :])
                    rbc = rbc_pool.tile([128, N], f32, tag="rbc", name=f"rbc{h}")
                    for ich in range(ICH):
                        isl = slice(ich * 512, (ich + 1) * 512)
                        rps = ps_mm.tile([128, 512], f32, tag="mm", name=f"rps{h}_{ich}")
                        nc.tensor.matmul(rps[:], ones_sb[:], rden[:, isl],
                                         start=True, stop=True)
                        nc.vector.tensor_copy(rbc[:, isl], rps[:])
                    nc.vector.tensor_mul(o_sb[:, h, :], otmps[h][0:64, :], rbc[0:64, :])

            attention_pair(0)
            attention_pair(1)

            # ---- output projection + bias/4, chunked AllReduce ----
            y_part = dram.tile([N, D], f32, tag="y_part")
            y_red = dram.tile([N, D], f32, tag="y_red")
            groups = [[0, 1, 2, 3], [4, 5, 6, 7]]

            RCH = 4  # allreduce chunks
            for rch in range(RCH):
                for ib in range(rch * JB // RCH, (rch + 1) * JB // RCH):
                    ibs = slice(ib * 128, (ib + 1) * 128)
                    for ch in range(D // 512):
                        yps = ps_mm.tile([128, 512], f32, tag="mm", name=f"yps{ib}_{ch}")
                        for h in range(HPC):
                            nc.tensor.matmul(
                                yps[:],
                                o_sb[:, h, ibs],
                                wo_sb[:, h, ch * 512:(ch + 1) * 512],
                                start=(h == 0), stop=(h == HPC - 1))
                        ysb = sb_work.tile([128, 512], f32, tag="y", name=f"ysb{ib}_{ch}")
                        nc.vector.tensor_add(ysb[:], yps[:], bias_bc[:, ch * 512:(ch + 1) * 512])
                        nc.sync.dma_start(out=y_part[ibs, ch * 512:(ch + 1) * 512], in_=ysb[:])
                nsl = slice(rch * (N // RCH), (rch + 1) * (N // RCH))
                nc.gpsimd.collective_compute(
                    "AllReduce",
                    mybir.AluOpType.add,
                    replica_groups=groups,
                    ins=[y_part[nsl, :]],
                    outs=[y_red[nsl, :]],
                )
                nc.sync.dma_start(out=y_out[nsl, :], in_=y_red[nsl, :])

    nc.finalize()
    return nc


def _make_in_maps(x, w_qkv, w_out, b_out):
    x = np.asarray(x, dtype=np.float32)
    w_qkv = np.asarray(w_qkv, dtype=np.float32)
    w_out = np.asarray(w_out, dtype=np.float32)
    b_out = np.asarray(b_out, dtype=np.float32)
    ones1 = np.ones((1, 128), dtype=np.float32)
    quart = np.full((1, 128), 0.25, dtype=np.float32)
    in_maps = []
    for c in range(NCORES):
        b = c // 4
        h0 = (c % 4) * HPC
        cols = np.arange(h0 * DH, (h0 + HPC) * DH)
        wq = w_qkv[:, cols]
        wk = w_qkv[:, D + cols]
        wv = w_qkv[:, 2 * D + cols]
        in_maps.append({
            "xT": np.ascontiguousarray(x[b].T),
            "wqk": np.ascontiguousarray(np.concatenate([wq, wk], axis=1)),
            "wv": np.ascontiguousarray(wv),
            "wout": np.ascontiguousarray(w_out[cols, :]),
            "bias": b_out[None, :],
            "ones1": ones1,
            "quart": quart,
        })
    return in_maps


def kernel(x, w_qkv, w_out, b_out):
    from concourse.bass_utils import run_bass_kernel_spmd

    if "nc" not in _cached:
        _cached["nc"] = _build_nc()
    nc = _cached["nc"]
    in_maps = _make_in_maps(x, w_qkv, w_out, b_out)
    res = run_bass_kernel_spmd(nc, in_maps, list(range(NCORES)))
    y0 = res.results[0]["y"]
    y1 = res.results[4]["y"]
    return np.stack([y0, y1], axis=0)


# revision 7
# speedup vs baseline: 1.4901x; 1.4901x over previous
"""Multi-head attention on 8 Trainium2 NeuronCores.

Problem: x[2, 2048, 1024] -> qkv proj (w_qkv [1024, 3072], 16 heads x 64) ->
softmax attention -> out proj (w_out [1024, 1024] + b_out).

Sharding: core c in 0..7 handles batch b = c // 4 and heads 4*(c%4) .. 4*(c%4)+3.
Each core computes a partial output projection over its 4 heads' slice; the four
cores of each batch group ReduceScatter(add) the partials chunk-by-chunk
(bias/4 folded in on every core), overlapped with later attention chunks. Core
g of a group ends up with rows [ich*512 + g*128, +128) of each chunk; the host
reassembles the full output from all 8 cores' shards.

Per-core dataflow (all matmuls fp32r: ~1e-4 rel err, bf16-rate on the PE):
  qkT [512, 2048] = wqk.T @ xT        (d-on-partitions layout for Q and K)
  v   [2048, 260] = xT.T @ wv         (natural layout, +ones column per head)
  then per query-chunk ich (512 rows), per head pair:
    S_T [j, i] = kT.T-slices @ qT     (scores transposed, K=64, head pairs
                                       share the PE via base-partition 0/64)
    attn_T = exp(S_T * 0.125)         (ScalarE, scale folded into activation)
    O_T[h] [65, 512] = v_aug.T @ attn_T   (row 64 = softmax denominator, free)
    o[h] = O_T[0:64] * (1/denom bcast)    (PE K=1 broadcast + DVE multiply)
    y[ich] += o[h].T @ wout[h] (+bias/4)  (K=64 per head, PSUM-accumulated)
    ReduceScatter(y[ich]) over the 4-core batch group.
"""

import numpy as np

N = 2048          # sequence length per batch
D = 1024          # model dim
DH = 64           # head dim
HPC = 4           # heads per core
NCORES = 8
GSIZE = 4         # cores per reduce group
SCALE = DH ** -0.5
ICH = N // 512    # query chunks

_cached = {}


def _build_nc():
    from contextlib import ExitStack

    import concourse.bacc as bacc
    import concourse.mybir as mybir
    from concourse import tile

    f32 = mybir.dt.float32
    f32r = mybir.dt.float32r

    nc = bacc.Bacc(num_devices=NCORES)

    xT = nc.declare_dram_parameter("xT", [D, N], f32r, isOutput=False)
    wqk = nc.declare_dram_parameter("wqk", [D, 2 * HPC * DH], f32r, isOutput=False)
    wv = nc.declare_dram_parameter("wv", [D, HPC * DH], f32r, isOutput=False)
    wout = nc.declare_dram_parameter("wout", [HPC * DH, D], f32r, isOutput=False)
    bias = nc.declare_dram_parameter("bias", [1, D], f32r, isOutput=False)
    ones1 = nc.declare_dram_parameter("ones1", [1, 128], f32r, isOutput=False)
    quart = nc.declare_dram_parameter("quart", [1, 128], f32r, isOutput=False)
    # per-core output: ICH shards of 128 rows (this core's ReduceScatter slices)
    y_out = nc.declare_dram_parameter("y", [ICH * 128, D], f32, isOutput=True)

    KB = D // 128           # 8 contraction blocks for the projections
    JB = N // 128           # 16 key blocks
    GRP = 2                 # j-blocks per exp group (PSUM: [128, 1024] tiles)
    VW = DH + 1             # v columns per head incl. ones column

    with tile.TileContext(nc) as tc:
        ctx = ExitStack()
        with ctx:
            sb = ctx.enter_context(tc.tile_pool(name="sb", bufs=1))
            ps_mm = ctx.enter_context(tc.tile_pool(name="ps_mm", bufs=2, space="PSUM"))
            ps_s = ctx.enter_context(tc.tile_pool(name="ps_s", bufs=2, space="PSUM"))
            ps_o = ctx.enter_context(tc.tile_pool(name="ps_o", bufs=2, space="PSUM"))
            dram = ctx.enter_context(tc.tile_pool(name="dram", bufs=1, space="DRAM"))

            # persistent SBUF residents (~105 KB/partition)
            qk_sb = sb.tile([128, 4, N], f32r, tag="qk")
            v_sb = sb.tile([128, JB, HPC * VW], f32r, tag="v")
            o_sb = sb.tile([64, HPC, N], f32r, tag="o")
            wo_sb = sb.tile([64, HPC, D], f32r, tag="wo")
            bias_bc = sb.tile([128, D], f32, tag="bias_bc")
            bias_sb = sb.tile([1, D], f32r, tag="bias")
            ones_sb = sb.tile([1, 128], f32r, tag="ones1")
            quart_sb = sb.tile([1, 128], f32r, tag="quart")

            nc.sync.dma_start(out=bias_sb[:], in_=bias[:, :])
            nc.sync.dma_start(out=ones_sb[:], in_=ones1[:, :])
            nc.sync.dma_start(out=quart_sb[:], in_=quart[:, :])
            for h in range(HPC):
                nc.sync.dma_start(out=wo_sb[:, h, :], in_=wout[h * DH:(h + 1) * DH, :])

            # bias/4 broadcast to 128 partitions (PE K=1 matmul)
            for ch in range(D // 512):
                bps = ps_mm.tile([128, 512], f32, tag="mm", name=f"bps{ch}")
                nc.tensor.matmul(bps[:], quart_sb[:], bias_sb[:, ch * 512:(ch + 1) * 512],
                                 start=True, stop=True)
                nc.vector.tensor_copy(bias_bc[:, ch * 512:(ch + 1) * 512], bps[:])

            # ---- stage 1: projections (xT + weights live only here) ----
            with ExitStack() as s1:
                sb_x = s1.enter_context(tc.tile_pool(name="sb_x", bufs=1))
                xT_sb = sb_x.tile([128, KB, N], f32r, tag="xT")
                for kb in range(KB):
                    nc.sync.dma_start(out=xT_sb[:, kb, :], in_=xT[kb * 128:(kb + 1) * 128, :])
                wqk_sb = sb_x.tile([128, KB, 2 * HPC * DH], f32r, tag="wqk")
                nc.sync.dma_start(out=wqk_sb[:], in_=wqk[:, :].rearrange("(kb p) m -> p kb m", p=128))
                wv_sb = sb_x.tile([128, KB, HPC * DH], f32r, tag="wv")
                nc.sync.dma_start(out=wv_sb[:], in_=wv[:, :].rearrange("(kb p) m -> p kb m", p=128))

                # qkT: mb 0 = q heads 01 | mb 1 = q heads 23 | mb 2 = k heads 01 | mb 3 = k heads 23
                for mb in [0, 2, 1, 3]:
                    for ich in range(ICH):
                        mps = ps_mm.tile([128, 512], f32, tag="mm", name=f"mps{mb}_{ich}")
                        for kb in range(KB):
                            nc.tensor.matmul(
                                mps[:],
                                wqk_sb[:, kb, mb * 128:(mb + 1) * 128],
                                xT_sb[:, kb, ich * 512:(ich + 1) * 512],
                                start=(kb == 0), stop=(kb == KB - 1))
                        nc.vector.tensor_copy(qk_sb[:, mb, ich * 512:(ich + 1) * 512], mps[:])

                # v natural; ones columns = whatever the evictions don't overwrite
                nc.vector.memset(v_sb[:].bitcast(f32), 1.0)
                for jb in range(JB):
                    vps = ps_mm.tile([128, 256], f32, tag="mm", name=f"vps{jb}")
                    for kb in range(KB):
                        nc.tensor.matmul(
                            vps[:],
                            xT_sb[:, kb, jb * 128:(jb + 1) * 128],
                            wv_sb[:, kb, :],
                            start=(kb == 0), stop=(kb == KB - 1))
                    nc.vector.tensor_copy(
                        v_sb[:, jb, :].rearrange("p (h c) -> p h c", c=VW)[:, :, 0:DH],
                        vps[:].rearrange("p (h c) -> p h c", c=DH))

            # ---- attention + projection + ReduceScatter, query-chunk-major ----
            sb_attn = ctx.enter_context(tc.tile_pool(name="sb_attn", bufs=3))
            sb_work = ctx.enter_context(tc.tile_pool(name="sb_work", bufs=2))
            otmp_pool = ctx.enter_context(tc.tile_pool(name="otmp", bufs=6))
            rbc_pool = ctx.enter_context(tc.tile_pool(name="rbc", bufs=2))

            y_part = dram.tile([N, D], f32, tag="y_part")
            y_red = dram.tile([ICH, 128, D], f32, tag="y_red")
            groups = [[0, 1, 2, 3], [4, 5, 6, 7]]

            for ich in range(ICH):
                isl = slice(ich * 512, (ich + 1) * 512)
                otmps = {}
                for pair in range(2):
                    ops = {}
                    for s in range(2):
                        h = pair * 2 + s
                        ops[h] = ps_o.tile([65, 512], f32, tag="o", name=f"ops{h}_{ich}")
                    for g in range(JB // GRP):
                        for s in range(2):
                            h = pair * 2 + s
                            psl = slice(s * 64, s * 64 + 64)
                            st = ps_s.tile([128, GRP * 512], f32, tag="s", name=f"st{h}_{ich}_{g}")
                            for u in range(GRP):
                                jb = g * GRP + u
                                nc.tensor.matmul(
                                    st[:, u * 512:(u + 1) * 512],
                                    qk_sb[psl, 2 + pair, jb * 128:(jb + 1) * 128],
                                    qk_sb[psl, pair, isl],
                                    start=True, stop=True)
                            at = sb_attn.tile([128, GRP * 512], f32r, tag="attn", name=f"at{h}_{ich}_{g}")
                            nc.scalar.activation(at[:], st[:],
                                                 mybir.ActivationFunctionType.Exp,
                                                 scale=float(SCALE))
                            for u in range(GRP):
                                jb = g * GRP + u
                                nc.tensor.matmul(
                                    ops[h][:],
                                    v_sb[:, jb, h * VW:(h + 1) * VW],
                                    at[:, u * 512:(u + 1) * 512],
                                    start=(g == 0 and u == 0),
                                    stop=(g == JB // GRP - 1 and u == GRP - 1))
                    for s in range(2):
                        h = pair * 2 + s
                        ot = otmp_pool.tile([65, 512], f32, tag="otmp", name=f"otmp{h}_{ich}")
                        nc.vector.tensor_copy(ot[:], ops[h][:])
                        otmps[h] = ot

                # normalize the 4 heads for this chunk
                for h in range(HPC):
                    rden = sb_work.tile([1, 512], f32r, tag="rden", name=f"rden{h}_{ich}")
                    with nc.allow_low_precision(reason="f32r rounding of softmax denom recip"):
                        nc.vector.reciprocal(rden[:], otmps[h][64:65, :])
                    rbc = rbc_pool.tile([128, 512], f32, tag="rbc", name=f"rbc{h}_{ich}")
                    rps = ps_mm.tile([128, 512], f32, tag="mm", name=f"rps{h}_{ich}")
                    nc.tensor.matmul(rps[:], ones_sb[:], rden[:], start=True, stop=True)
                    nc.vector.tensor_copy(rbc[:], rps[:])
                    nc.vector.tensor_mul(o_sb[:, h, isl], otmps[h][0:64, :], rbc[0:64, :])

                # output projection for this chunk + bias/4
                for ib in range(ich * 4, (ich + 1) * 4):
                    ibs = slice(ib * 128, (ib + 1) * 128)
                    for ch in range(D // 512):
                        yps = ps_mm.tile([128, 512], f32, tag="mm", name=f"yps{ib}_{ch}")
                        for h in range(HPC):
                            nc.tensor.matmul(
                                yps[:],
                                o_sb[:, h, ibs],
                                wo_sb[:, h, ch * 512:(ch + 1) * 512],
                                start=(h == 0), stop=(h == HPC - 1))
                        ysb = sb_work.tile([128, 512], f32, tag="y", name=f"ysb{ib}_{ch}")
                        nc.vector.tensor_add(ysb[:], yps[:], bias_bc[:, ch * 512:(ch + 1) * 512])
                        nc.sync.dma_start(out=y_part[ibs, ch * 512:(ch + 1) * 512], in_=ysb[:])

                # ReduceScatter this chunk across the batch group
                nc.gpsimd.collective_compute(
                    "ReduceScatter",
                    mybir.AluOpType.add,
                    replica_groups=groups,
                    ins=[y_part[isl, :]],
                    outs=[y_red[ich]],
                )

            # ship the shards (after all collectives so no engine stalls mid-pipeline)
            for ich in range(ICH):
                nc.gpsimd.dma_start(out=y_out[ich * 128:(ich + 1) * 128, :], in_=y_red[ich])

    nc.finalize()
    return nc


def _make_in_maps(x, w_qkv, w_out, b_out):
    x = np.asarray(x, dtype=np.float32)
    w_qkv = np.asarray(w_qkv, dtype=np.float32)
    w_out = np.asarray(w_out, dtype=np.float32)
    b_out = np.asarray(b_out, dtype=np.float32)
    ones1 = np.ones((1, 128), dtype=np.float32)
    quart = np.full((1, 128), 0.25, dtype=np.float32)
    in_maps = []
    for c in range(NCORES):
        b = c // GSIZE
        h0 = (c % GSIZE) * HPC
        cols = np.arange(h0 * DH, (h0 + HPC) * DH)
        wq = w_qkv[:, cols]
        wk = w_qkv[:, D + cols]
        wv = w_qkv[:, 2 * D + cols]
        in_maps.append({
            "xT": np.ascontiguousarray(x[b].T),
            "wqk": np.ascontiguousarray(np.concatenate([wq, wk], axis=1)),
            "wv": np.ascontiguousarray(wv),
            "wout": np.ascontiguousarray(w_out[cols, :]),
            "bias": b_out[None, :],
            "ones1": ones1,
            "quart": quart,
        })
    return in_maps


def _assemble(results, x_shape):
    B = x_shape[0]
    y = np.empty((B, N, D), dtype=np.float32)
    for b in range(B):
        for g in range(GSIZE):
            shard = results[b * GSIZE + g]["y"]  # [ICH*128, D]
            for ich in range(ICH):
                y[b, ich * 512 + g * 128: ich * 512 + (g + 1) * 128, :] = \
                    shard[ich * 128:(ich + 1) * 128, :]
    return y


def kernel(x, w_qkv, w_out, b_out):
    from concourse.bass_utils import run_bass_kernel_spmd

    if "nc" not in _cached:
        _cached["nc"] = _build_nc()
    nc = _cached["nc"]
    in_maps = _make_in_maps(x, w_qkv, w_out, b_out)
    res = run_bass_kernel_spmd(nc, in_maps, list(range(NCORES)))
    return _assemble(res.results, np.asarray(x).shape)
